# revision 17
# baseline (speedup 1.0000x reference)
"""Segment-mean (sentence pooling) Bass/Tile kernel for Trainium2.

Problem: last_hidden_state [16, 4096, 1024] f32, sentence_mask [16, 4096] int,
num_sents=32. For each (batch, sentence id): mean of hidden states at seq
positions whose mask equals the id. Returns (embeddings [16, 32, 1024] f32,
unique_sents [32] int).

Strategy: data-parallel over batch across 8 NeuronCores (2 batches/core).
On each core, per batch: stream seq-chunks of hidden states via DMA and
compute the segment sum as a matmul against a host-built one-hot mask chunk
[128, 32] (stationary lhsT), accumulating over chunks into PSUM [32, 512]x2.
Memory-bound: 32 MiB/core of hidden states.

Modes:
- "split16" (default): exact-fp32 result at bf16-class PE cost. Each f32
  chunk is split on-chip into hi = fp16(x) (ScalarE cast) and
  lo = fp16(x - hi) (VectorE fused sub+cast); both fp16 matmuls accumulate
  into the same PSUM group. The one-hot weights are 0/1 in fp16 (exact), and
  the 1/count scaling is applied on the PSUM->SBUF copy (per-partition
  tensor_scalar multiply). fp16 splitting carries 22 mantissa bits; error is
  at the fp32-accumulation-envelope level (~1e-7 relative).
- "f32": plain fp32 matmul (exact but 4 PE cycles/row; PE-bound).
- "f32r": fp32r matmul (fast but ~2e-4 relative error).
"""

import numpy as np

BATCH, SEQ, HID, NS = 16, 4096, 1024, 32
P = 128
NCORES = 8
BPC = BATCH // NCORES  # batches per core
CHUNKS = SEQ // P  # seq chunks of 128
CPD = 2  # chunks per DMA (1 MiB DMAs)

MODE = "split16"
HBUFS = 10  # hidden-tile buffer depth (x 1 MiB)

_nc_cache = {}


def _make_tile_context(tile_mod, nc):
    """TileContext whose exit skips the on-device semaphore clears + second
    all-engine barrier: the NEFF epilogue already zeroes every semaphore, so
    the Tile-level RANGE_CLEAR round trip (~2us) is redundant."""
    from concourse.vector_clock import ScopedClock

    class FastTileContext(tile_mod.TileContext):
        def _drain_and_barrier(self, tick_clock, wait_clock):
            drain_inst = self.nc.sync.drain()
            wait_clock.add_sem_waits(
                drain_inst.ins, ScopedClock({None: tick_clock.global_clock})
            )
            self.nc.all_engine_barrier()
            popped = self.nc._tile_sem_poison_stack.pop()
            assert popped is self._sem_poison
            sems = list(self.sems.allocated().values())
            sem_nums = [s.num if hasattr(s, "num") else s for s in sems]
            self.nc._state.prepend_free_semaphores(sem_nums)
            for poison_set in self.nc._tile_sem_poison_stack:
                poison_set.update(sem_nums)

    return FastTileContext(nc)


def _build_nc(mode):
    import concourse.mybir as mybir
    import concourse.tile as tile
    from concourse import bacc

    nc = bacc.Bacc(
        "TRN2", target_bir_lowering=False, debug=False, enable_asserts=False
    )
    f32 = mybir.dt.float32
    f16 = mybir.dt.float16

    h = nc.dram_tensor("h", [BPC, SEQ, HID], f32, kind="ExternalInput")
    o = nc.dram_tensor("o", [BPC, NS, HID], f32, kind="ExternalOutput")
    w_dt = f16 if mode == "split16" else f32
    w = nc.dram_tensor("w", [BPC, P, CHUNKS, NS], w_dt, kind="ExternalInput")
    if mode == "split16":
        invc = nc.dram_tensor("invc", [BPC, NS, 1], f32, kind="ExternalInput")
    # [b, g, p, q, d]: DMA tile g covers chunks (2g, 2g+1); q = chunk-in-DMA
    h5 = h.ap().rearrange("b (g q p) d -> b g p q d", q=CPD, p=P)

    sb_dt = mybir.dt.float32r if mode == "f32r" else f32

    with _make_tile_context(tile, nc) as tc:
        with (
            tc.tile_pool(name="wp", bufs=1) as wp,
            tc.tile_pool(name="hp", bufs=HBUFS) as hp,
            tc.tile_pool(name="sp", bufs=8) as sp,
            tc.tile_pool(name="op", bufs=2) as op,
            tc.tile_pool(name="pp", bufs=2 * BPC, space="PSUM") as pp,
        ):
            if mode == "split16":
                # warm the ACT Copy table off the critical path: the first
                # ACTIVATE pays a ~1.5us ACT_TABLE_LOAD
                warm = wp.tile([P, 8], mybir.dt.float32, tag="warm", name="warm")
                nc.vector.memset(warm[:], 0.0)
                warm16 = wp.tile([P, 8], f16, tag="warm16", name="warm16")
                nc.scalar.copy(warm16[:], warm[:])
            wts, invcs = [], []
            for b in range(BPC):
                wt = wp.tile([P, CHUNKS, NS], w_dt, tag=f"w{b}", name=f"w{b}")
                if mode == "f32r":
                    nc.gpsimd.dma_start(wt[:], w.ap()[b])  # casts f32 -> f32r
                else:
                    # scalar HWDGE ring: keep the SP ring a pure h-stream
                    nc.scalar.dma_start(wt[:], w.ap()[b])
                wts.append(wt)
                if mode == "split16":
                    ic = wp.tile([NS, 1], f32, tag=f"ic{b}", name=f"ic{b}")
                    nc.scalar.dma_start(ic[:], invc.ap()[b])
                    invcs.append(ic)

            def group_plan(b):
                return [(g * CPD, CPD) for g in range(CHUNKS // CPD)]

            h3 = h.ap().rearrange("b (c p) d -> b c p d", p=P)
            for b in range(BPC):
                ps = [
                    pp.tile([NS, 512], f32, tag=f"ps{n}", name=f"ps{b}_{n}")
                    for n in range(2)
                ]
                for c0, gw in group_plan(b):
                    ht = hp.tile([P, gw, HID], sb_dt, tag="h", name=f"h{b}_{c0}")
                    src = h3[b, c0 : c0 + gw].rearrange("c p d -> p c d")
                    if mode == "f32r":
                        nc.gpsimd.dma_start(ht[:], src)
                    else:
                        nc.sync.dma_start(ht[:], src)
                    if mode == "split16":
                        # group-wide hi/lo split (amortizes ACT/DVE op overhead)
                        hi = sp.tile([P, gw, HID], f16, tag="hi", name=f"hi{b}_{c0}")
                        nc.scalar.copy(hi[:], ht[:])
                        lo = sp.tile([P, gw, HID], f16, tag="lo", name=f"lo{b}_{c0}")
                        nc.vector.tensor_sub(lo[:], ht[:], hi[:])
                        for q in range(gw):
                            c = c0 + q
                            for part, pfirst in ((hi, True), (lo, False)):
                                for n in range(2):
                                    nc.tensor.matmul(
                                        ps[n][:],
                                        wts[b][:, c, :],
                                        part[:, q, n * 512 : (n + 1) * 512],
                                        start=(c == 0 and pfirst),
                                        stop=(c == CHUNKS - 1 and not pfirst),
                                    )
                    else:
                        for q in range(gw):
                            c = c0 + q
                            for n in range(2):
                                nc.tensor.matmul(
                                    ps[n][:],
                                    wts[b][:, c, :],
                                    ht[:, q, n * 512 : (n + 1) * 512],
                                    start=(c == 0),
                                    stop=(c == CHUNKS - 1),
                                )
                ot = op.tile([NS, HID], f32, tag="o", name=f"o{b}")
                for n in range(2):
                    if mode == "split16":
                        nc.vector.tensor_scalar_mul(
                            ot[:, n * 512 : (n + 1) * 512], ps[n][:], invcs[b][:]
                        )
                    else:
                        nc.vector.tensor_copy(ot[:, n * 512 : (n + 1) * 512], ps[n][:])
                nc.scalar.dma_start(o.ap()[b], ot[:])
    nc.compile()
    return nc


def _get_nc(mode=None):
    mode = mode or MODE
    if mode not in _nc_cache:
        _nc_cache[mode] = _build_nc(mode)
    return _nc_cache[mode]


def _prepare(last_hidden_state, sentence_mask, num_sents, mode=None):
    """Host prep: shard on batch, build the one-hot mask tensor."""
    mode = mode or MODE
    lhs = np.ascontiguousarray(np.asarray(last_hidden_state, dtype=np.float32))
    mask = np.asarray(sentence_mask)
    ns = int(num_sents)
    assert lhs.shape == (BATCH, SEQ, HID) and ns == NS, (lhs.shape, ns)

    ids = mask.astype(np.int64)
    onehot = ids[:, :, None] == np.arange(ns, dtype=np.int64)[None, None, :]
    counts = onehot.sum(axis=1)  # [B, NS]
    inv = (1.0 / np.maximum(counts, 1)).astype(np.float32)
    if mode == "split16":
        w = onehot.astype(np.float16)  # 0/1, exact
    else:
        w = onehot.astype(np.float32) * inv[:, None, :]
    # [B, S, NS] -> [B, P, CHUNKS, NS] with S = c*P + p
    w = np.ascontiguousarray(w.reshape(BATCH, CHUNKS, P, ns).transpose(0, 2, 1, 3))

    in_maps = []
    for i in range(NCORES):
        m = {
            "h": lhs[i * BPC : (i + 1) * BPC],
            "w": w[i * BPC : (i + 1) * BPC],
        }
        if mode == "split16":
            m["invc"] = np.ascontiguousarray(
                inv[i * BPC : (i + 1) * BPC, :, None]
            )
        in_maps.append(m)
    return in_maps, mask.dtype


def _execute(in_maps, trace=False, mode=None, **kwargs):
    from concourse.bass_utils import run_bass_kernel_spmd

    return run_bass_kernel_spmd(
        _get_nc(mode),
        in_maps,
        core_ids=list(range(NCORES)),
        trace=trace,
        **kwargs,
    )


def _gather(results):
    return np.concatenate([r["o"] for r in results], axis=0)


def kernel(last_hidden_state, sentence_mask, num_sents):
    in_maps, mask_dtype = _prepare(last_hidden_state, sentence_mask, num_sents)
    res = _execute(in_maps)
    emb = _gather(res.results)
    unique_sents = np.arange(int(num_sents), dtype=mask_dtype)
    return emb, unique_sents


# revision 32
# speedup vs baseline: 1.1249x; 1.1249x over previous
"""Segment-mean (sentence pooling) Bass/Tile kernel for Trainium2.

Problem: last_hidden_state [16, 4096, 1024] f32, sentence_mask [16, 4096] int,
num_sents=32. For each (batch, sentence id): mean of hidden states at seq
positions whose mask equals the id. Returns (embeddings [16, 32, 1024] f32,
unique_sents [32] int).

Strategy: data-parallel over batch across 8 NeuronCores (2 batches/core).
On each core, per batch: stream seq-chunks of hidden states via DMA and
compute the segment sum as a matmul against a host-built one-hot mask chunk
[128, 32] (stationary lhsT), accumulating over chunks into PSUM [32, 512]x2.
Memory-bound: 32 MiB/core of hidden states.

Modes:
- "split16" (default): exact-fp32 result at bf16-class PE cost. Each f32
  chunk is split on-chip into hi = fp16(x) (ScalarE cast) and
  lo = fp16(x - hi) (VectorE fused sub+cast); both fp16 matmuls accumulate
  into the same PSUM group. The one-hot weights are 0/1 in fp16 (exact), and
  the 1/count scaling is applied on the PSUM->SBUF copy (per-partition
  tensor_scalar multiply). fp16 splitting carries 22 mantissa bits; error is
  at the fp32-accumulation-envelope level (~1e-7 relative).
- "f32": plain fp32 matmul (exact but 4 PE cycles/row; PE-bound).
- "f32r": fp32r matmul (fast but ~2e-4 relative error).
"""

import numpy as np

BATCH, SEQ, HID, NS = 16, 4096, 1024, 32
P = 128
NCORES = 8
BPC = BATCH // NCORES  # batches per core
CHUNKS = SEQ // P  # seq chunks of 128
CPD = 2  # chunks per DMA (1 MiB DMAs)

MODE = "split16"
HBUFS = 10  # hidden-tile buffer depth (x 1 MiB)

_nc_cache = {}


def _make_tile_context(tile_mod, nc):
    """TileContext whose exit skips the on-device semaphore clears + second
    all-engine barrier: the NEFF epilogue already zeroes every semaphore, so
    the Tile-level RANGE_CLEAR round trip (~2us) is redundant."""
    from concourse.vector_clock import ScopedClock

    class FastTileContext(tile_mod.TileContext):
        def _drain_and_barrier(self, tick_clock, wait_clock):
            drain_inst = self.nc.sync.drain()
            wait_clock.add_sem_waits(
                drain_inst.ins, ScopedClock({None: tick_clock.global_clock})
            )
            self.nc.all_engine_barrier()
            popped = self.nc._tile_sem_poison_stack.pop()
            assert popped is self._sem_poison
            sems = list(self.sems.allocated().values())
            sem_nums = [s.num if hasattr(s, "num") else s for s in sems]
            self.nc._state.prepend_free_semaphores(sem_nums)
            for poison_set in self.nc._tile_sem_poison_stack:
                poison_set.update(sem_nums)

    return FastTileContext(nc)


def _build_nc(mode):
    import concourse.mybir as mybir
    import concourse.tile as tile
    from concourse import bacc

    nc = bacc.Bacc(
        "TRN2", target_bir_lowering=False, debug=False, enable_asserts=False
    )
    f32 = mybir.dt.float32
    f16 = mybir.dt.float16

    h = nc.dram_tensor("h", [BPC, SEQ, HID], f32, kind="ExternalInput")
    o = nc.dram_tensor("o", [BPC, NS, HID], f32, kind="ExternalOutput")
    w_dt = f16 if mode == "split16" else f32
    w = nc.dram_tensor("w", [BPC, P, CHUNKS, NS], w_dt, kind="ExternalInput")
    if mode == "split16":
        invc = nc.dram_tensor("invc", [BPC, NS, 1], f32, kind="ExternalInput")
    # [b, g, p, q, d]: DMA tile g covers chunks (2g, 2g+1); q = chunk-in-DMA
    h5 = h.ap().rearrange("b (g q p) d -> b g p q d", q=CPD, p=P)

    sb_dt = mybir.dt.float32r if mode == "f32r" else f32

    with _make_tile_context(tile, nc) as tc:
        with (
            tc.tile_pool(name="wp", bufs=1) as wp,
            tc.tile_pool(name="hp", bufs=HBUFS) as hp,
            tc.tile_pool(name="sp", bufs=8) as sp,
            tc.tile_pool(name="op", bufs=2) as op,
            tc.tile_pool(name="pp", bufs=2 * BPC, space="PSUM") as pp,
        ):
            if mode == "split16":
                # warm the ACT Copy table off the critical path: the first
                # ACTIVATE pays a ~1.5us ACT_TABLE_LOAD
                warm = wp.tile([P, 8], mybir.dt.float32, tag="warm", name="warm")
                nc.vector.memset(warm[:], 0.0)
                warm16 = wp.tile([P, 8], f16, tag="warm16", name="warm16")
                nc.scalar.copy(warm16[:], warm[:])
            wts, invcs = [], []
            for b in range(BPC):
                wt = wp.tile([P, CHUNKS, NS], w_dt, tag=f"w{b}", name=f"w{b}")
                if mode == "f32r":
                    nc.gpsimd.dma_start(wt[:], w.ap()[b])  # casts f32 -> f32r
                else:
                    # scalar HWDGE ring: keep the SP ring a pure h-stream
                    nc.scalar.dma_start(wt[:], w.ap()[b])
                wts.append(wt)
                if mode == "split16":
                    ic = wp.tile([NS, 1], f32, tag=f"ic{b}", name=f"ic{b}")
                    nc.scalar.dma_start(ic[:], invc.ap()[b])
                    invcs.append(ic)

            def group_plan(b):
                return [(g * CPD, CPD) for g in range(CHUNKS // CPD)]

            h3 = h.ap().rearrange("b (c p) d -> b c p d", p=P)
            for b in range(BPC):
                ps = [
                    pp.tile([NS, 512], f32, tag=f"ps{n}", name=f"ps{b}_{n}")
                    for n in range(2)
                ]
                for c0, gw in group_plan(b):
                    ht = hp.tile([P, gw, HID], sb_dt, tag="h", name=f"h{b}_{c0}")
                    src = h5[b, c0 // CPD]
                    if mode == "f32r":
                        nc.gpsimd.dma_start(ht[:], src)
                    else:
                        nc.sync.dma_start(ht[:], src)
                    if mode == "split16":
                        # group-wide hi/lo split (amortizes ACT/DVE op overhead)
                        hi = sp.tile([P, gw, HID], f16, tag="hi", name=f"hi{b}_{c0}")
                        nc.scalar.copy(hi[:], ht[:])
                        lo = sp.tile([P, gw, HID], f16, tag="lo", name=f"lo{b}_{c0}")
                        nc.vector.tensor_sub(lo[:], ht[:], hi[:])
                        for q in range(gw):
                            c = c0 + q
                            for part, pfirst in ((hi, True), (lo, False)):
                                for n in range(2):
                                    nc.tensor.matmul(
                                        ps[n][:],
                                        wts[b][:, c, :],
                                        part[:, q, n * 512 : (n + 1) * 512],
                                        start=(c == 0 and pfirst),
                                        stop=(c == CHUNKS - 1 and not pfirst),
                                    )
                    else:
                        for q in range(gw):
                            c = c0 + q
                            for n in range(2):
                                nc.tensor.matmul(
                                    ps[n][:],
                                    wts[b][:, c, :],
                                    ht[:, q, n * 512 : (n + 1) * 512],
                                    start=(c == 0),
                                    stop=(c == CHUNKS - 1),
                                )
                ot = op.tile([NS, HID], f32, tag="o", name=f"o{b}")
                for n in range(2):
                    if mode == "split16":
                        nc.vector.tensor_scalar_mul(
                            ot[:, n * 512 : (n + 1) * 512], ps[n][:], invcs[b][:]
                        )
                    else:
                        nc.vector.tensor_copy(ot[:, n * 512 : (n + 1) * 512], ps[n][:])
                nc.scalar.dma_start(o.ap()[b], ot[:])
    nc.compile()
    return nc


def _get_nc(mode=None):
    mode = mode or MODE
    if mode not in _nc_cache:
        _nc_cache[mode] = _build_nc(mode)
    return _nc_cache[mode]


def _prepare(last_hidden_state, sentence_mask, num_sents, mode=None):
    """Host prep: shard on batch, build the one-hot mask tensor."""
    mode = mode or MODE
    lhs = np.ascontiguousarray(np.asarray(last_hidden_state, dtype=np.float32))
    mask = np.asarray(sentence_mask)
    ns = int(num_sents)
    assert lhs.shape == (BATCH, SEQ, HID) and ns == NS, (lhs.shape, ns)

    ids = mask.astype(np.int64)
    onehot = ids[:, :, None] == np.arange(ns, dtype=np.int64)[None, None, :]
    counts = onehot.sum(axis=1)  # [B, NS]
    inv = (1.0 / np.maximum(counts, 1)).astype(np.float32)
    if mode == "split16":
        w = onehot.astype(np.float16)  # 0/1, exact
    else:
        w = onehot.astype(np.float32) * inv[:, None, :]
    # [B, S, NS] -> [B, P, CHUNKS, NS] with S = c*P + p
    w = np.ascontiguousarray(w.reshape(BATCH, CHUNKS, P, ns).transpose(0, 2, 1, 3))

    in_maps = []
    for i in range(NCORES):
        m = {
            "h": lhs[i * BPC : (i + 1) * BPC],
            "w": w[i * BPC : (i + 1) * BPC],
        }
        if mode == "split16":
            m["invc"] = np.ascontiguousarray(
                inv[i * BPC : (i + 1) * BPC, :, None]
            )
        in_maps.append(m)
    return in_maps, mask.dtype


def _execute(in_maps, trace=False, mode=None, **kwargs):
    from concourse.bass_utils import run_bass_kernel_spmd

    return run_bass_kernel_spmd(
        _get_nc(mode),
        in_maps,
        core_ids=list(range(NCORES)),
        trace=trace,
        **kwargs,
    )


def _gather(results):
    return np.concatenate([r["o"] for r in results], axis=0)


def kernel(last_hidden_state, sentence_mask, num_sents):
    in_maps, mask_dtype = _prepare(last_hidden_state, sentence_mask, num_sents)
    res = _execute(in_maps)
    emb = _gather(res.results)
    unique_sents = np.arange(int(num_sents), dtype=mask_dtype)
    return emb, unique_sents
